# revision 7
# baseline (speedup 1.0000x reference)
"""Distributed AND-convolution (Dempster combination / FWHT-style) for 8 TRN2 cores.

out = mobius(zeta(m1) * zeta(m2)) over 24 bit-axes, L = 2^24.

Sharding: top 3 bits (h = k[23:21]) = core id. Per core per channel: 2^21
elements, SBUF (128, 16384): partition p = l[20:14], free f = l[13:0].

v2 pipeline (overlap-first):
 - slabs = 4 contiguous 4096-col blocks; fwd order 3,2,0,1-load with A2As
   emitted 3,2,1,0 so the first collective fires ~55us in (vs ~190 in v1).
 - cross-slab zeta stages (free bits 13,12) are woven in as whole-slab
   adds at the right points (S2+=S3, S1+=S3, S0+=S2orig, S0+=S1cur).
 - ALL inverse work except the pc-bit (k[20:18]) mobius runs BEFORE the
   back A2A (free-bit stages / c+pm mobius commute with pc mobius), so
   the post-back-A2A tail is just kron(M3,I16) matmul + output DMA.
 - back A2As merged into 2x4MiB (slabs {3,2} and {1,0}).
 - collective train order: F3 F2 F1 B32 F0 B10.
"""
import sys
sys.path.insert(0, '/opt/trn_rl_repo')
import numpy as np

NCORES = 8
P = 128
F = 16384
BLK = 512            # matmul block
NBLK = F // BLK      # 32
NSLAB = 4
MSL = NBLK // NSLAB  # 8 blocks per slab
SLAB = F // NSLAB    # 4096 contiguous cols


def _zeta_mat(nbits):
    idx = np.arange(1 << nbits)
    return ((idx[:, None] & idx[None, :]) == idx[None, :]).astype(np.float32)


def _mobius_mat(nbits):
    idx = np.arange(1 << nbits)
    sup = (idx[:, None] & idx[None, :]) == idx[None, :]
    pc = np.array([bin(x).count("1") for x in range(1 << nbits)])
    signs = (-1.0) ** pc[idx[:, None] & ~idx[None, :]]
    return (sup * signs).astype(np.float32)


def build_kernel():
    import concourse.bacc as bacc
    import concourse.tile as tile
    from concourse import mybir

    f32 = mybir.dt.float32
    nc = bacc.Bacc("TRN2", target_bir_lowering=False, debug=False, num_devices=NCORES)

    m1_in = nc.dram_tensor("m1", [P, F], f32, kind="ExternalInput")
    m2_in = nc.dram_tensor("m2", [P, F], f32, kind="ExternalInput")
    out_t = nc.dram_tensor("out", [P, F], f32, kind="ExternalOutput")

    WZ7_d = nc.inline_tensor(_zeta_mat(7), name="WZ7")
    WZ3_d = nc.inline_tensor(np.kron(_zeta_mat(3), np.eye(16, dtype=np.float32)), name="WZ3x")
    # c-bit + pm-bit mobius (pre back-A2A, partitions = (c, pm))
    WM34_d = nc.inline_tensor(np.kron(_mobius_mat(3), _mobius_mat(4)), name="WM34")
    # flo-bit (free bits 6:0) mobius for the Fb transpose pass
    WM7F_d = nc.inline_tensor(_mobius_mat(7), name="WM7F")
    # pc-bit mobius (post back-A2A, partitions = (pc, pm))
    WM3I_d = nc.inline_tensor(np.kron(_mobius_mat(3), np.eye(16, dtype=np.float32)), name="WM3I")

    with tile.TileContext(nc) as tc:
        with tc.tile_pool(name="sbuf", bufs=1) as pool, \
             tc.tile_pool(name="chunks", bufs=4) as cpool, \
             tc.tile_pool(name="psum", bufs=2, space="PSUM") as psum, \
             tc.tile_pool(name="dram", bufs=1, space="DRAM") as dram:

            wz7 = pool.tile([P, P], f32)
            wz3 = pool.tile([P, P], f32)
            wm34 = pool.tile([P, P], f32)
            wm7f = pool.tile([P, P], f32)
            wm3i = pool.tile([P, P], f32)
            nc.sync.dma_start(out=wz7[:], in_=WZ7_d[:])
            nc.sync.dma_start(out=wz3[:], in_=WZ3_d[:])
            nc.sync.dma_start(out=wm34[:], in_=WM34_d[:])
            nc.sync.dma_start(out=wm7f[:], in_=WM7F_d[:])
            nc.sync.dma_start(out=wm3i[:], in_=WM3I_d[:])

            A = pool.tile([P, F], f32)
            B = pool.tile([P, F], f32)

            cc_in = [dram.tile([NCORES, 2, 16, SLAB], f32, tag=f"cci{s}", name=f"cci{s}") for s in range(NSLAB)]
            cc_out = [dram.tile([NCORES, 2, 16, SLAB], f32, tag=f"cco{s}", name=f"cco{s}") for s in range(NSLAB)]
            # back A2As: 2 of them, each carrying two slabs (1 channel)
            cc2_in = [dram.tile([NCORES, 16, 2 * SLAB], f32, tag=f"c2i{g}", name=f"c2i{g}") for g in range(2)]
            cc2_out = [dram.tile([NCORES, 16, 2 * SLAB], f32, tag=f"c2o{g}", name=f"c2o{g}") for g in range(2)]

            def cols(s):
                return (s * SLAB, (s + 1) * SLAB)

            def tt(alu, lo, hi):
                f = getattr(nc.vector, "tensor_add" if alu == "add" else "tensor_sub")
                f(lo, lo, hi)

            def slab_add(t, sd, ss, alu="add"):
                c0, c1 = cols(sd)
                d0, d1 = cols(ss)
                tt(alu, t[:, c0:c1], t[:, d0:d1])

            def slab_stages(t, s, alu):
                # intra-slab free-bit stages j=0..11 on contiguous slab s
                c0, c1 = cols(s)
                sl = t[:, c0:c1]
                for j in range(12):
                    w = sl.rearrange("p (a two b) -> p a two b", two=2, b=1 << j)
                    tt(alu, w[:, :, 0, :], w[:, :, 1, :])

            def mm_block(t, w, blk):
                # t[:, blk] = w.T @ t[:, blk] ; evac via ACT
                ps = psum.tile([P, BLK], f32, tag="ps_mm")
                sl = t[:, blk * BLK:(blk + 1) * BLK]
                nc.tensor.matmul(ps[:], lhsT=w[:], rhs=sl, start=True, stop=True)
                nc.scalar.copy(sl, ps[:])

            def load_slab(t, src, s):
                c0, c1 = cols(s)
                nc.sync.dma_start(out=t[:, c0:c1], in_=src[:, c0:c1])

            def fwd_local(s):
                # per-slab local fwd work for both channels: free-bit stages
                # THEN the partition-bit zeta, so every slab the weave adds
                # touch is in the same (jstaged + WZ7) state.
                for t in (A, B):
                    slab_stages(t, s, "add")
                    for m in range(MSL):
                        mm_block(t, wz7, s * MSL + m)

            def stage_fwd(s):
                c0, c1 = cols(s)
                for d in range(NCORES):
                    nc.sync.dma_start(out=cc_in[s][d, 0], in_=A[16 * d:16 * (d + 1), c0:c1])
                    nc.sync.dma_start(out=cc_in[s][d, 1], in_=B[16 * d:16 * (d + 1), c0:c1])
                nc.gpsimd.collective_compute(
                    "AllToAll", mybir.AluOpType.bypass,
                    replica_groups=[list(range(NCORES))],
                    ins=[cc_in[s][:].opt()], outs=[cc_out[s][:].opt()],
                )

            def mid_slab(s):
                # recv fwd A2A, h-conv: zeta3 on both channels, product into A.
                # Then the inverse transform via two fused transpose-matmul
                # passes on TensorE: Fa applies mobius(c,pm) (partition dim)
                # while transposing each 128-chunk into B; Fb applies
                # mobius(flo = free bits 6:0) while transposing back into A.
                # Remaining free bits 11:7 via 5 DVE stages. (Bits 13:12 are
                # the cross-slab weave; pc bits get WM3I post back-A2A.)
                c0, c1 = cols(s)
                for c in range(NCORES):
                    nc.gpsimd.dma_start(out=A[16 * c:16 * (c + 1), c0:c1], in_=cc_out[s][c, 0])
                    nc.gpsimd.dma_start(out=B[16 * c:16 * (c + 1), c0:c1], in_=cc_out[s][c, 1])
                for m in range(MSL):
                    blk = s * MSL + m
                    sa = A[:, blk * BLK:(blk + 1) * BLK]
                    sb = B[:, blk * BLK:(blk + 1) * BLK]
                    psA = psum.tile([P, BLK], f32, tag="psA")
                    psB = psum.tile([P, BLK], f32, tag="psB")
                    nc.tensor.matmul(psA[:], lhsT=wz3[:], rhs=sa, start=True, stop=True)
                    nc.tensor.matmul(psB[:], lhsT=wz3[:], rhs=sb, start=True, stop=True)
                    qa = cpool.tile([P, BLK], f32, tag="qa")
                    nc.scalar.copy(qa[:], psA[:])
                    nc.vector.tensor_mul(sa, qa[:], psB[:])
                # Fa: A chunks -> (mobius(c,pm))^T-transformed transposed chunks in B
                for m in range(MSL):
                    blk = s * MSL + m
                    psFa = psum.tile([P, BLK], f32, tag="psA")
                    for j in range(4):
                        ch = blk * BLK + j * P
                        nc.tensor.matmul(psFa[:, j * P:(j + 1) * P],
                                         lhsT=A[:, ch:ch + P], rhs=wm34[:],
                                         start=True, stop=True)
                    nc.scalar.copy(B[:, blk * BLK:(blk + 1) * BLK], psFa[:])
                # Fb: B chunks -> flo-mobius + transpose back into A
                for m in range(MSL):
                    blk = s * MSL + m
                    psFb = psum.tile([P, BLK], f32, tag="psB")
                    for j in range(4):
                        ch = blk * BLK + j * P
                        nc.tensor.matmul(psFb[:, j * P:(j + 1) * P],
                                         lhsT=B[:, ch:ch + P], rhs=wm7f[:],
                                         start=True, stop=True)
                    nc.scalar.copy(A[:, blk * BLK:(blk + 1) * BLK], psFb[:])
                # inverse free-bit stages j=7..11 (flo 6:0 done by Fb)
                sl = A[:, c0:c1]
                for j in range(7, 12):
                    w = sl.rearrange("p (a two b) -> p a two b", two=2, b=1 << j)
                    tt("sub", w[:, :, 0, :], w[:, :, 1, :])

            def stage_back(g, slabs):
                # stage two inv'd slabs + back A2A
                for d in range(NCORES):
                    for i, s in enumerate(slabs):
                        c0, c1 = cols(s)
                        nc.sync.dma_start(out=cc2_in[g][d, :, i * SLAB:(i + 1) * SLAB],
                                          in_=A[16 * d:16 * (d + 1), c0:c1])
                nc.gpsimd.collective_compute(
                    "AllToAll", mybir.AluOpType.bypass,
                    replica_groups=[list(range(NCORES))],
                    ins=[cc2_in[g][:].opt()], outs=[cc2_out[g][:].opt()],
                )

            def tail_slabs(g, slabs):
                # recv back A2A into B, pc-mobius matmul, stream out
                for i, s in enumerate(slabs):
                    c0, c1 = cols(s)
                    for d in range(NCORES):
                        nc.gpsimd.dma_start(out=B[16 * d:16 * (d + 1), c0:c1],
                                            in_=cc2_out[g][d, :, i * SLAB:(i + 1) * SLAB])
                    for m in range(MSL):
                        mm_block(B, wm3i, s * MSL + m)
                    nc.sync.dma_start(out=out_t[:, c0:c1], in_=B[:, c0:c1])

            # ---------------- forward pipeline ----------------
            # ALL forward work is emitted before any mid work: engine queues
            # execute in order, so a mid op waiting on an A2A result must not
            # sit ahead of independent fwd work in the DVE/ACT/TE queues.
            for t, src in ((A, m1_in), (B, m2_in)):
                load_slab(t, src, 3)
            fwd_local(3)
            stage_fwd(3)                       # F3

            for t, src in ((A, m1_in), (B, m2_in)):
                load_slab(t, src, 2)
            fwd_local(2)
            for t in (A, B):
                slab_add(t, 2, 3)              # S2 += S3
            stage_fwd(2)                       # F2

            for t, src in ((A, m1_in), (B, m2_in)):
                load_slab(t, src, 1)
            fwd_local(1)
            for t in (A, B):
                slab_add(t, 1, 3)              # S1 += S3
            stage_fwd(1)                       # F1

            for t, src in ((A, m1_in), (B, m2_in)):
                load_slab(t, src, 0)
            fwd_local(0)
            for t in (A, B):
                slab_add(t, 0, 1)              # S0 += (S1+S3)
                slab_add(t, 0, 2)              # S0 += (S2+S3)
                slab_add(t, 0, 3, "sub")       # S0 -= S3  => S0+S1+S2+S3
            stage_fwd(0)                       # F0

            mid_slab(3)
            mid_slab(2)
            # inv weave for slabs 3,2: S2 -= S3 (slab3/2 fully inv'd)
            slab_add(A, 2, 3, "sub")
            stage_back(0, (3, 2))              # B32

            mid_slab(1)
            mid_slab(0)
            # inv weave: S1 -= S3 ; S0 -= S1cur ; S0 -= S2cur ; S0 -= S3
            slab_add(A, 1, 3, "sub")
            slab_add(A, 0, 1, "sub")
            slab_add(A, 0, 2, "sub")
            slab_add(A, 0, 3, "sub")
            stage_back(1, (1, 0))              # B10

            tail_slabs(0, (3, 2))
            tail_slabs(1, (1, 0))

    nc.compile()
    return nc


_NC_CACHE = None


def kernel(m12: np.ndarray) -> np.ndarray:
    global _NC_CACHE
    from concourse.bass_utils import run_bass_kernel_spmd

    if _NC_CACHE is None:
        _NC_CACHE = build_kernel()
    nc = _NC_CACHE

    m12 = np.ascontiguousarray(np.asarray(m12, dtype=np.float32))
    Bsz, C, L = m12.shape
    S = L // NCORES
    in_maps = []
    for c in range(NCORES):
        in_maps.append({
            "m1": m12[0, 0, c * S:(c + 1) * S].reshape(P, F),
            "m2": m12[0, 1, c * S:(c + 1) * S].reshape(P, F),
        })
    try:
        res = run_bass_kernel_spmd(nc, in_maps, core_ids=list(range(NCORES)))
    except Exception:
        # transient NRT/device hiccups have been observed; retry once
        import time
        time.sleep(5)
        res = run_bass_kernel_spmd(nc, in_maps, core_ids=list(range(NCORES)))
    out = np.concatenate([res.results[c]["out"].reshape(-1) for c in range(NCORES)])
    return out.reshape(1, L, 1, 1)


if __name__ == "__main__":
    m12 = np.load("/root/problem/m12.npy")
    out = kernel(m12)
    exp = np.load("/root/problem/expected.npy")
    err = np.abs(out - exp).max()
    scale = np.abs(exp).max()
    print(f"absmax err {err:.4g} scale {scale:.4g} rel {err/scale:.3e}")


# revision 11
# speedup vs baseline: 1.0363x; 1.0363x over previous
"""Distributed AND-convolution (Dempster combination / FWHT-style) for 8 TRN2 cores.

out = mobius(zeta(m1) * zeta(m2)) over 24 bit-axes, L = 2^24.

Sharding: top 3 bits (h = k[23:21]) = core id. Per core per channel: 2^21
elements, SBUF (128, 16384): partition p = l[20:14], free f = l[13:0].

v2 pipeline (overlap-first):
 - slabs = 4 contiguous 4096-col blocks; fwd order 3,2,0,1-load with A2As
   emitted 3,2,1,0 so the first collective fires ~55us in (vs ~190 in v1).
 - cross-slab zeta stages (free bits 13,12) are woven in as whole-slab
   adds at the right points (S2+=S3, S1+=S3, S0+=S2orig, S0+=S1cur).
 - ALL inverse work except the pc-bit (k[20:18]) mobius runs BEFORE the
   back A2A (free-bit stages / c+pm mobius commute with pc mobius), so
   the post-back-A2A tail is just kron(M3,I16) matmul + output DMA.
 - back A2As merged into 2x4MiB (slabs {3,2} and {1,0}).
 - collective train order: F3 F2 F1 B32 F0 B10.
"""
import sys
sys.path.insert(0, '/opt/trn_rl_repo')
import numpy as np

NCORES = 8
P = 128
F = 16384
BLK = 512            # matmul block
NBLK = F // BLK      # 32
NSLAB = 4
MSL = NBLK // NSLAB  # 8 blocks per slab
SLAB = F // NSLAB    # 4096 contiguous cols


def _zeta_mat(nbits):
    idx = np.arange(1 << nbits)
    return ((idx[:, None] & idx[None, :]) == idx[None, :]).astype(np.float32)


def _mobius_mat(nbits):
    idx = np.arange(1 << nbits)
    sup = (idx[:, None] & idx[None, :]) == idx[None, :]
    pc = np.array([bin(x).count("1") for x in range(1 << nbits)])
    signs = (-1.0) ** pc[idx[:, None] & ~idx[None, :]]
    return (sup * signs).astype(np.float32)


def build_kernel():
    import concourse.bacc as bacc
    import concourse.tile as tile
    from concourse import mybir

    f32 = mybir.dt.float32
    nc = bacc.Bacc("TRN2", target_bir_lowering=False, debug=False, num_devices=NCORES)

    m1_in = nc.dram_tensor("m1", [P, F], f32, kind="ExternalInput")
    m2_in = nc.dram_tensor("m2", [P, F], f32, kind="ExternalInput")
    out_t = nc.dram_tensor("out", [P, F], f32, kind="ExternalOutput")

    WZ7_d = nc.inline_tensor(_zeta_mat(7), name="WZ7")
    WZ3_d = nc.inline_tensor(np.kron(_zeta_mat(3), np.eye(16, dtype=np.float32)), name="WZ3x")
    # c-bit + pm-bit mobius (pre back-A2A, partitions = (c, pm))
    WM34_d = nc.inline_tensor(np.kron(_mobius_mat(3), _mobius_mat(4)), name="WM34")
    # flo-bit (free bits 6:0) mobius for the Fb transpose pass
    WM7F_d = nc.inline_tensor(_mobius_mat(7), name="WM7F")
    # pc-bit mobius (post back-A2A, partitions = (pc, pm))
    WM3I_d = nc.inline_tensor(np.kron(_mobius_mat(3), np.eye(16, dtype=np.float32)), name="WM3I")

    with tile.TileContext(nc) as tc:
        with tc.tile_pool(name="sbuf", bufs=1) as pool, \
             tc.tile_pool(name="chunks", bufs=4) as cpool, \
             tc.tile_pool(name="psum", bufs=2, space="PSUM") as psum, \
             tc.tile_pool(name="dram", bufs=1, space="DRAM") as dram:

            wz7 = pool.tile([P, P], f32)
            wz3 = pool.tile([P, P], f32)
            wm34 = pool.tile([P, P], f32)
            wm7f = pool.tile([P, P], f32)
            wm3i = pool.tile([P, P], f32)
            nc.sync.dma_start(out=wz7[:], in_=WZ7_d[:])
            nc.sync.dma_start(out=wz3[:], in_=WZ3_d[:])
            nc.sync.dma_start(out=wm34[:], in_=WM34_d[:])
            nc.sync.dma_start(out=wm7f[:], in_=WM7F_d[:])
            nc.sync.dma_start(out=wm3i[:], in_=WM3I_d[:])

            A = pool.tile([P, F], f32)
            B = pool.tile([P, F], f32)

            cc_in = [dram.tile([NCORES, 2, 16, SLAB], f32, tag=f"cci{s}", name=f"cci{s}") for s in range(NSLAB)]
            cc_out = [dram.tile([NCORES, 2, 16, SLAB], f32, tag=f"cco{s}", name=f"cco{s}") for s in range(NSLAB)]
            # back A2As: 2 of them, each carrying two slabs (1 channel)
            cc2_in = [dram.tile([NCORES, 16, 2 * SLAB], f32, tag=f"c2i{g}", name=f"c2i{g}") for g in range(2)]
            cc2_out = [dram.tile([NCORES, 16, 2 * SLAB], f32, tag=f"c2o{g}", name=f"c2o{g}") for g in range(2)]

            def cols(s):
                return (s * SLAB, (s + 1) * SLAB)

            def tt(alu, lo, hi):
                f = getattr(nc.vector, "tensor_add" if alu == "add" else "tensor_sub")
                f(lo, lo, hi)

            def slab_add(t, sd, ss, alu="add"):
                # two half-ops so consecutive adds on one slab form two
                # shorter dependency chains (hides DVE sem latency)
                c0, _ = cols(sd)
                d0, _ = cols(ss)
                for h in (0, 1):
                    tt(alu, t[:, c0 + h * 2048:c0 + (h + 1) * 2048],
                       t[:, d0 + h * 2048:d0 + (h + 1) * 2048])

            def slab_stages(t, s, alu):
                # intra-slab free-bit stages j=0..11 on contiguous slab s
                c0, c1 = cols(s)
                sl = t[:, c0:c1]
                for j in range(12):
                    w = sl.rearrange("p (a two b) -> p a two b", two=2, b=1 << j)
                    tt(alu, w[:, :, 0, :], w[:, :, 1, :])

            def fwd_stages_pair(s):
                # stages j=0..11 for BOTH channels with the per-op dependency
                # chains interleaved 4-ways (channel x half-slab): each DVE op
                # carries ~1.7us of semaphore latency on its predecessor, so a
                # single 12-deep chain is latency-bound. j=0..10 act within
                # 2048-col halves; j=11 crosses them once.
                c0, c1 = cols(s)
                for j in range(11):
                    for t in (A, B):
                        for h in (0, 1):
                            sl = t[:, c0 + h * 2048:c0 + (h + 1) * 2048]
                            w = sl.rearrange("p (a two b) -> p a two b", two=2, b=1 << j)
                            tt("add", w[:, :, 0, :], w[:, :, 1, :])
                for t in (A, B):
                    sl = t[:, c0:c1]
                    w = sl.rearrange("p (a two b) -> p a two b", two=2, b=1 << 11)
                    tt("add", w[:, :, 0, :], w[:, :, 1, :])

            def mm_block(t, w, blk):
                # t[:, blk] = w.T @ t[:, blk] ; evac via ACT
                ps = psum.tile([P, BLK], f32, tag="ps_mm")
                sl = t[:, blk * BLK:(blk + 1) * BLK]
                nc.tensor.matmul(ps[:], lhsT=w[:], rhs=sl, start=True, stop=True)
                nc.scalar.copy(sl, ps[:])

            def load_slab(t, src, s):
                c0, c1 = cols(s)
                nc.sync.dma_start(out=t[:, c0:c1], in_=src[:, c0:c1])

            def fwd_local(s):
                # per-slab local fwd work for both channels: free-bit stages
                # THEN the partition-bit zeta, so every slab the weave adds
                # touch is in the same (jstaged + WZ7) state.
                fwd_stages_pair(s)
                for t in (A, B):
                    for m in range(MSL):
                        mm_block(t, wz7, s * MSL + m)

            def stage_fwd(s):
                c0, c1 = cols(s)
                for d in range(NCORES):
                    nc.sync.dma_start(out=cc_in[s][d, 0], in_=A[16 * d:16 * (d + 1), c0:c1])
                    nc.sync.dma_start(out=cc_in[s][d, 1], in_=B[16 * d:16 * (d + 1), c0:c1])
                nc.gpsimd.collective_compute(
                    "AllToAll", mybir.AluOpType.bypass,
                    replica_groups=[list(range(NCORES))],
                    ins=[cc_in[s][:].opt()], outs=[cc_out[s][:].opt()],
                )

            def mid_slab(s):
                # recv fwd A2A, h-conv: zeta3 on both channels, product into A.
                # Then the inverse transform via two fused transpose-matmul
                # passes on TensorE: Fa applies mobius(c,pm) (partition dim)
                # while transposing each 128-chunk into B; Fb applies
                # mobius(flo = free bits 6:0) while transposing back into A.
                # Remaining free bits 11:7 via 5 DVE stages. (Bits 13:12 are
                # the cross-slab weave; pc bits get WM3I post back-A2A.)
                c0, c1 = cols(s)
                for c in range(NCORES):
                    nc.gpsimd.dma_start(out=A[16 * c:16 * (c + 1), c0:c1], in_=cc_out[s][c, 0])
                    nc.gpsimd.dma_start(out=B[16 * c:16 * (c + 1), c0:c1], in_=cc_out[s][c, 1])
                for m in range(MSL):
                    blk = s * MSL + m
                    sa = A[:, blk * BLK:(blk + 1) * BLK]
                    sb = B[:, blk * BLK:(blk + 1) * BLK]
                    psA = psum.tile([P, BLK], f32, tag="psA")
                    psB = psum.tile([P, BLK], f32, tag="psB")
                    nc.tensor.matmul(psA[:], lhsT=wz3[:], rhs=sa, start=True, stop=True)
                    nc.tensor.matmul(psB[:], lhsT=wz3[:], rhs=sb, start=True, stop=True)
                    qa = cpool.tile([P, BLK], f32, tag="qa")
                    nc.scalar.copy(qa[:], psA[:])
                    nc.vector.tensor_mul(sa, qa[:], psB[:])
                # Fa: A chunks -> (mobius(c,pm))^T-transformed transposed chunks in B
                for m in range(MSL):
                    blk = s * MSL + m
                    psFa = psum.tile([P, BLK], f32, tag="psA")
                    for j in range(4):
                        ch = blk * BLK + j * P
                        nc.tensor.matmul(psFa[:, j * P:(j + 1) * P],
                                         lhsT=A[:, ch:ch + P], rhs=wm34[:],
                                         start=True, stop=True)
                    nc.scalar.copy(B[:, blk * BLK:(blk + 1) * BLK], psFa[:])
                # Fb: B chunks -> flo-mobius + transpose back into A
                for m in range(MSL):
                    blk = s * MSL + m
                    psFb = psum.tile([P, BLK], f32, tag="psB")
                    for j in range(4):
                        ch = blk * BLK + j * P
                        nc.tensor.matmul(psFb[:, j * P:(j + 1) * P],
                                         lhsT=B[:, ch:ch + P], rhs=wm7f[:],
                                         start=True, stop=True)
                    nc.scalar.copy(A[:, blk * BLK:(blk + 1) * BLK], psFb[:])
                # inverse free-bit stages j=7..11 (flo 6:0 done by Fb),
                # halves interleaved to hide DVE sem latency
                for j in range(7, 11):
                    for h in (0, 1):
                        sl = A[:, c0 + h * 2048:c0 + (h + 1) * 2048]
                        w = sl.rearrange("p (a two b) -> p a two b", two=2, b=1 << j)
                        tt("sub", w[:, :, 0, :], w[:, :, 1, :])
                sl = A[:, c0:c1]
                w = sl.rearrange("p (a two b) -> p a two b", two=2, b=1 << 11)
                tt("sub", w[:, :, 0, :], w[:, :, 1, :])

            def stage_back(g, slabs):
                # stage two inv'd slabs + back A2A
                for d in range(NCORES):
                    for i, s in enumerate(slabs):
                        c0, c1 = cols(s)
                        nc.sync.dma_start(out=cc2_in[g][d, :, i * SLAB:(i + 1) * SLAB],
                                          in_=A[16 * d:16 * (d + 1), c0:c1])
                nc.gpsimd.collective_compute(
                    "AllToAll", mybir.AluOpType.bypass,
                    replica_groups=[list(range(NCORES))],
                    ins=[cc2_in[g][:].opt()], outs=[cc2_out[g][:].opt()],
                )

            def tail_slabs(g, slabs):
                # recv back A2A into B, pc-mobius matmul, stream out
                for i, s in enumerate(slabs):
                    c0, c1 = cols(s)
                    for d in range(NCORES):
                        nc.gpsimd.dma_start(out=B[16 * d:16 * (d + 1), c0:c1],
                                            in_=cc2_out[g][d, :, i * SLAB:(i + 1) * SLAB])
                    for m in range(MSL):
                        mm_block(B, wm3i, s * MSL + m)
                    nc.sync.dma_start(out=out_t[:, c0:c1], in_=B[:, c0:c1])

            # ---------------- forward pipeline ----------------
            # ALL forward work is emitted before any mid work: engine queues
            # execute in order, so a mid op waiting on an A2A result must not
            # sit ahead of independent fwd work in the DVE/ACT/TE queues.
            for t, src in ((A, m1_in), (B, m2_in)):
                load_slab(t, src, 3)
            fwd_local(3)
            stage_fwd(3)                       # F3

            for t, src in ((A, m1_in), (B, m2_in)):
                load_slab(t, src, 2)
            fwd_local(2)
            for t in (A, B):
                slab_add(t, 2, 3)              # S2 += S3
            stage_fwd(2)                       # F2

            for t, src in ((A, m1_in), (B, m2_in)):
                load_slab(t, src, 1)
            fwd_local(1)
            for t in (A, B):
                slab_add(t, 1, 3)              # S1 += S3
            stage_fwd(1)                       # F1

            for t, src in ((A, m1_in), (B, m2_in)):
                load_slab(t, src, 0)
            fwd_local(0)
            for t in (A, B):
                slab_add(t, 0, 1)              # S0 += (S1+S3)
                slab_add(t, 0, 2)              # S0 += (S2+S3)
                slab_add(t, 0, 3, "sub")       # S0 -= S3  => S0+S1+S2+S3
            stage_fwd(0)                       # F0

            mid_slab(3)
            mid_slab(2)
            # inv weave for slabs 3,2: S2 -= S3 (slab3/2 fully inv'd)
            slab_add(A, 2, 3, "sub")
            stage_back(0, (3, 2))              # B32

            mid_slab(1)
            mid_slab(0)
            # inv weave: S1 -= S3 ; S0 -= S1cur ; S0 -= S2cur ; S0 -= S3
            slab_add(A, 1, 3, "sub")
            slab_add(A, 0, 1, "sub")
            slab_add(A, 0, 2, "sub")
            slab_add(A, 0, 3, "sub")
            stage_back(1, (1, 0))              # B10

            tail_slabs(0, (3, 2))
            tail_slabs(1, (1, 0))

    nc.compile()
    return nc


_NC_CACHE = None


def kernel(m12: np.ndarray) -> np.ndarray:
    global _NC_CACHE
    from concourse.bass_utils import run_bass_kernel_spmd

    if _NC_CACHE is None:
        _NC_CACHE = build_kernel()
    nc = _NC_CACHE

    m12 = np.ascontiguousarray(np.asarray(m12, dtype=np.float32))
    Bsz, C, L = m12.shape
    S = L // NCORES
    in_maps = []
    for c in range(NCORES):
        in_maps.append({
            "m1": m12[0, 0, c * S:(c + 1) * S].reshape(P, F),
            "m2": m12[0, 1, c * S:(c + 1) * S].reshape(P, F),
        })
    try:
        res = run_bass_kernel_spmd(nc, in_maps, core_ids=list(range(NCORES)))
    except Exception:
        # transient NRT/device hiccups have been observed; retry once
        import time
        time.sleep(5)
        res = run_bass_kernel_spmd(nc, in_maps, core_ids=list(range(NCORES)))
    out = np.concatenate([res.results[c]["out"].reshape(-1) for c in range(NCORES)])
    return out.reshape(1, L, 1, 1)


if __name__ == "__main__":
    m12 = np.load("/root/problem/m12.npy")
    out = kernel(m12)
    exp = np.load("/root/problem/expected.npy")
    err = np.abs(out - exp).max()
    scale = np.abs(exp).max()
    print(f"absmax err {err:.4g} scale {scale:.4g} rel {err/scale:.3e}")
